# revision 22
# baseline (speedup 1.0000x reference)
"""Trainium2 Bass kernel for nn_CQLoss (composite loss function).

Strategy: pure data parallel over batch dim (64 batches -> 8 per core).

All large tensors travel as fp8 e4m3 (TRN FP8_EXP4 == ml_dtypes.float8_e4m3):
the kernel is DMA-bound and the DMA cost is out-side bytes, so fp8 halves the
bf16 baseline's traffic.  Quantization contributes ~2e-3 relative error on
the final scalar (tolerance 2e-2).

Per core:
  - d = (rz|pts)[mapping] - (zs|pts_gt): the host packs [-zs | -pts_gt] rows
    pre-transposed to the exact SBUF layout [S, BL*K] (plain DMA pairs
    elements linearly in AP order, so identical shapes on both sides make
    the pairing trivially correct), and [rzs | pts] as the gather source.
    neg is DMA'd in plainly (HWDGE); the indirect gather lands on top with
    compute_op=add, so the subtraction happens inside the DMA engines.
  - sum-of-squares via TensorE gram trick: matmul(chunk, chunk) accumulated
    into a PSUM region; the diagonal of the accumulated gram holds
    sum-over-partitions of squares per column, so trace == sum d^2.  All
    regions live in ONE psum bank as ONE accumulation group (hw `start`
    zeroes the bank, so interleaved groups in a bank clobber each other;
    a single group accumulating at different addresses is safe):
    recon [*,0:128], pts [*,128:256], kl [*,256:384], markx [*,384:512]
    (landmark P-indices host-permuted to the front of the PC block; markx
    grams cols D..D+16 of every batch, mask selects the 8 mark cols).
  - KL: qt = V*qy (fp8, SBUF layout), l = Ln(qt + V*eps) on ScalarE (fp8
    out), then gram(qt_chunk, l_chunk): trace == sum qt*ln(qt).
  - best: f32 via cpack, sqrt(w)-prescaled, DVE sub + ScalarE Square with
    accum_out (tiny).
  - extraction: one DVE tensor_mul of the psum bank against an f32
    weight-diagonal mask (term normalizations baked in, DMA'd during the
    tail shadow), then one tensor_scalar accumulate into an acc column;
    host sums in f64.

Raw bass (explicit semaphores): standalone wait_ge ops, cumulative counts on
per-stream semaphores (DMAs of one stream complete in FIFO order), explicit
self-waits for same-engine RAW pairs.
"""

import os
import sys

import numpy as np

for _p in ("/opt/trn_rl_repo", "/root/.axon_site/_ro/trn_rl_repo"):
    if os.path.isdir(_p) and _p not in sys.path:
        sys.path.insert(0, _p)

B, S, D, P, C, V = 64, 128, 2048, 118, 2, 512
PC = P * C  # 236
K = D + PC  # combined row width: 2284
N_CORES = 8
BL = B // N_CORES  # 8 batches per core
ALPHA, BETA, GAMMA, EPS = 10.0, 0.1, 1.0, 1e-20
MARKS = (0, 29, 88, 117)
NMARK = len(MARKS)  # 4 -> 8 cols (C=2), host-permuted to the front
W_MARK = ALPHA * PC / (NMARK * C)  # 295.0

# neg-chunk layout: (start_batch, n_batches); small leading chunks so the
# first gathers can start early
NCH = [(0, 1), (1, 1), (2, 2), (4, 2), (6, 2)]


def _chunk_of(b):
    return next(i for i, (bs, nb) in enumerate(NCH) if bs <= b < bs + nb)


# cpack layout (f32 cols): 0..7 mapping (int32 bits), 8 ln bias,
# 9..24 sqrt(w)*best, 25..40 sqrt(w)*best_gt
BC = BL * C  # 16
COL_MAP = 0
COL_BIAS = BL  # 8
COL_BEST = 9
COL_BESTGT = 9 + BC  # 25
NCONST = 9 + 2 * BC  # 41

# psum region column offsets (single bank, single accumulation group)
R_RECON, R_PTS, R_KL, R_MARK = 0, 128, 256, 384

# DoubleRow fp8 matmul: two 128-col k-tiles per pass (2x PE throughput)
USE_DR = True

# sync-engine DMA program: ("cp",) ("neg",chunk) ("qt",half) ("wd",)
# ("wg",val) = wait sem_g >= val, to let gather transfers interleave
SCHED = [
    ("neg", 0),
    ("neg", 1),
    ("cp",),
    ("neg", 2),
    ("neg", 3),
    ("neg", 4),
    ("qt", 0),
    ("qt", 1),
    ("wd",),
]

_CACHE: dict = {}


def _build_bass(vector_dims: int):
    import concourse.bass as bass
    from concourse import mybir

    f32 = mybir.dt.float32
    fp8 = mybir.dt.float8e4
    i32 = mybir.dt.int32
    Act = mybir.ActivationFunctionType
    Alu = mybir.AluOpType

    nc = bass.Bass()

    neg = nc.dram_tensor("neg", [S, BL * K], fp8, kind="ExternalInput")
    gath = nc.dram_tensor("gath", [BL * S, K], fp8, kind="ExternalInput")
    qt = nc.dram_tensor("qt", [S, BL * V], fp8, kind="ExternalInput")
    cpack = nc.dram_tensor("cpack", [S, NCONST], f32, kind="ExternalInput")
    wdiag = nc.dram_tensor("wdiag", [S, 512], f32, kind="ExternalInput")
    # acc col 0: weighted recon+pts+kl+marks; col 1 (rows 0:118): best
    po = nc.dram_tensor("po", [S, 2], f32, kind="ExternalOutput")

    from contextlib import ExitStack

    with ExitStack() as ctx:
        d_t = ctx.enter_context(nc.sbuf_tensor([S, BL * K], fp8))
        qt_t = ctx.enter_context(nc.sbuf_tensor([S, BL * V], fp8))
        l_t = ctx.enter_context(nc.sbuf_tensor([S, BL * V], fp8))
        cp_t = ctx.enter_context(nc.sbuf_tensor([S, NCONST], f32))
        wd_t = ctx.enter_context(nc.sbuf_tensor([S, 512], f32))
        bd_t = ctx.enter_context(nc.sbuf_tensor([S, BC], f32))
        ext_t = ctx.enter_context(nc.sbuf_tensor([S, 512], f32))
        acc_t = ctx.enter_context(nc.sbuf_tensor([S, 2], f32))
        ps = ctx.enter_context(nc.psum_tensor([S, 512], f32))

        sem_cp = ctx.enter_context(nc.semaphore("sem_cp"))
        sem_neg = ctx.enter_context(nc.semaphore("sem_neg"))
        sem_qt = ctx.enter_context(nc.semaphore("sem_qt"))
        sem_g = ctx.enter_context(nc.semaphore("sem_g"))
        sem_wd = ctx.enter_context(nc.semaphore("sem_wd"))
        sem_act = ctx.enter_context(nc.semaphore("sem_act"))
        sem_dve = ctx.enter_context(nc.semaphore("sem_dve"))
        sem_pe = ctx.enter_context(nc.semaphore("sem_pe"))
        sem_out = ctx.enter_context(nc.semaphore("sem_out"))
        block = ctx.enter_context(nc.Block(no_gpsimd_drain=True))

        d3 = d_t[:].rearrange("s (b k) -> s b k", b=BL)
        map_i = cp_t[:, COL_MAP : COL_MAP + BL].bitcast(i32)

        @block.sync
        def _(sync):
            qh = BL // 2 * V
            for tok in SCHED:
                kind = tok[0]
                if kind == "cp":
                    sync.dma_start(out=cp_t[:], in_=cpack[:]).then_inc(sem_cp, 16)
                elif kind == "neg":
                    bs, nb = NCH[tok[1]]
                    sync.dma_start(
                        out=d_t[:, bs * K : (bs + nb) * K],
                        in_=neg[:, bs * K : (bs + nb) * K],
                    ).then_inc(sem_neg, 16)
                elif kind == "qt":
                    sl = slice(0, qh) if tok[1] == 0 else slice(qh, BL * V)
                    sync.dma_start(out=qt_t[:, sl], in_=qt[:, sl]).then_inc(
                        sem_qt, 16
                    )
                elif kind == "wd":
                    sync.dma_start(out=wd_t[:], in_=wdiag[:]).then_inc(sem_wd, 16)
                elif kind == "wg":
                    sync.wait_ge(sem_g, tok[1])
            sync.wait_ge(sem_dve, 2)
            sync.wait_ge(sem_act, 3)
            sync.dma_start(out=po[:], in_=acc_t[:]).then_inc(sem_out, 16)
            sync.wait_ge(sem_out, 16)

        @block.gpsimd
        def _(gpsimd):
            gpsimd.wait_ge(sem_cp, 16)  # mapping loaded
            for b in range(BL):
                gpsimd.wait_ge(sem_neg, 16 * (_chunk_of(b) + 1))
                gpsimd.indirect_dma_start(
                    out=d_t[:, b * K : (b + 1) * K],
                    out_offset=None,
                    in_=gath[:],
                    in_offset=bass.IndirectOffsetOnAxis(
                        ap=map_i[:, b : b + 1], axis=0
                    ),
                    compute_op=Alu.add,
                ).then_inc(sem_g, 16)

        # single accumulation group across the whole bank: only the very
        # first matmul has start=True, only the very last has stop=True
        first_mm = [True]
        DR = mybir.MatmulPerfMode.DoubleRow

        def mm(out, lhsT, rhs, stop=False, dr=False):
            inst = nc.tensor.matmul(
                out=out,
                lhsT=lhsT,
                rhs=rhs,
                start=first_mm[0],
                stop=stop,
                perf_mode=DR if dr else None,
                skip_group_check=True,
            )
            first_mm[0] = False
            return inst

        def gram(region, a, b=None):
            # square-gram of columns `a` (or cross-gram a x b) into `region`;
            # USE_DR pairs two k-tiles per pass (DoubleRow)
            if USE_DR:
                w = a.shape[-1] // 2
                a2 = a.rearrange("s (t c) -> s t c", t=2)
                b2 = a2 if b is None else b.rearrange("s (t c) -> s t c", t=2)
                return mm(ps[0:w, region : region + w], a2, b2, dr=True)
            w = a.shape[-1]
            return mm(ps[0:w, region : region + w], a, a if b is None else b)

        def d_grams(tensor, b):
            tensor.wait_ge(sem_g, 16 * (b + 1))
            rw = 256 if USE_DR else 128
            for i in range(D // rw):
                gram(R_RECON, d3[:, b, i * rw : (i + 1) * rw])
            if USE_DR:
                gram(R_PTS, d3[:, b, D : D + PC])
            else:
                gram(R_PTS, d3[:, b, D : D + 128])
                gram(R_PTS, d3[:, b, D + 128 : D + PC])
            # marks: first 16 cols of the pts block (8 landmark + 8 other);
            # wdiag keeps only p < 8.  rhs must have a single free dim, so
            # one small gram per batch, all accumulating into [16,16].
            return mm(
                ps[0:16, R_MARK : R_MARK + 16],
                d3[:, b, D : D + 16],
                d3[:, b, D : D + 16],
                stop=(b == BL - 1),
            )

        @block.tensor
        def _(tensor):
            for b in range(6):
                d_grams(tensor, b)
            # kl grams: ln halves are done by the time batch 5 is gathered
            kw = 256 if USE_DR else 128
            tensor.wait_ge(sem_act, 2)
            for c in range(BL * V // kw):
                gram(
                    R_KL,
                    qt_t[:, c * kw : (c + 1) * kw],
                    l_t[:, c * kw : (c + 1) * kw],
                )
            for b in range(6, BL):
                last = d_grams(tensor, b)
            last.then_inc(sem_pe, 1)

        @block.scalar
        def _(scalar):
            scalar.wait_ge(sem_cp, 16)
            scalar.wait_ge(sem_qt, 16)
            nc.scalar.activation(
                l_t[:, : BL // 2 * V],
                qt_t[:, : BL // 2 * V],
                Act.Ln,
                bias=cp_t[:, COL_BIAS : COL_BIAS + 1],
                scale=1.0,
            ).then_inc(sem_act, 1)
            scalar.wait_ge(sem_qt, 32)
            nc.scalar.activation(
                l_t[:, BL // 2 * V :],
                qt_t[:, BL // 2 * V :],
                Act.Ln,
                bias=cp_t[:, COL_BIAS : COL_BIAS + 1],
                scale=1.0,
            ).then_inc(sem_act, 1)
            # best term: acc[0:118, 1] = per-partition sum(bd^2), in place
            scalar.wait_ge(sem_dve, 1)
            nc.scalar.activation(
                bd_t[0:P, :],
                bd_t[0:P, :],
                Act.Square,
                accum_out=acc_t[0:P, 1:2],
            ).then_inc(sem_act, 1)

        @block.vector
        def _(vector):
            # best diff: bd = sqrt(w)*(best - best_gt), f32
            vector.wait_ge(sem_cp, 16)
            nc.vector.tensor_sub(
                bd_t[:],
                cp_t[:, COL_BEST : COL_BEST + BC],
                cp_t[:, COL_BESTGT : COL_BESTGT + BC],
            ).then_inc(sem_dve, 1)  # 1
            # fused extraction: (psum * 1.0) * wdiag, per-partition accumulate
            vector.wait_ge(sem_wd, 16)
            vector.wait_ge(sem_pe, 1)
            nc.vector.scalar_tensor_tensor(
                out=ext_t[:],
                in0=ps[:, :],
                scalar=1.0,
                in1=wd_t[:],
                op0=Alu.mult,
                op1=Alu.mult,
                accum_out=acc_t[:, 0:1],
            ).then_inc(sem_dve, 1)  # 2

    return nc


def _get_nc(vector_dims: int):
    key = ("nc", vector_dims)
    if key not in _CACHE:
        _CACHE[key] = _build_bass(vector_dims)
    return _CACHE[key]


def _prepare(inputs):
    import ml_dtypes

    fp8 = ml_dtypes.float8_e4m3

    zs = np.asarray(inputs["zs"], dtype=np.float32)
    rzs = np.asarray(inputs["rzs"], dtype=np.float32)
    pts = np.asarray(inputs["pts"], dtype=np.float32)
    pts_gt = np.asarray(inputs["pts_gt"], dtype=np.float32)
    qy = np.asarray(inputs["qy"], dtype=np.float32)
    best = np.asarray(inputs["best"], dtype=np.float64)
    best_gt = np.asarray(inputs["best_gt"], dtype=np.float64)
    mapping = np.asarray(inputs["mapping"])
    vector_dims = int(np.asarray(inputs["vector_dims"]))

    # landmark P-indices permuted to the front of the P axis
    perm = list(MARKS) + [p for p in range(P) if p not in MARKS]
    pts_p = pts[:, :, perm, :].reshape(B, S, PC)
    ptsgt_p = pts_gt[:, :, perm, :].reshape(B, S, PC)

    neg_b = np.empty((B, S, K), dtype=fp8)
    neg_b[:, :, :D] = (-zs).astype(fp8)
    neg_b[:, :, D:] = (-ptsgt_p).astype(fp8)
    gath_b = np.empty((B, S, K), dtype=fp8)
    gath_b[:, :, :D] = rzs.astype(fp8)
    gath_b[:, :, D:] = pts_p.astype(fp8)
    qt_b = (qy * vector_dims).astype(fp8)

    # sqrt of landmark weights for the best term (exact in f64)
    w_p = np.ones(P, dtype=np.float64)
    w_p[list(MARKS)] += W_MARK
    w_sq = np.sqrt(w_p)
    best_w = (best * w_sq[None, :, None]).astype(np.float32)
    bestgt_w = (best_gt * w_sq[None, :, None]).astype(np.float32)

    # weighted diagonal extraction mask (term normalizations baked in)
    wd = np.zeros((S, 512), dtype=np.float32)
    ii = np.arange(128)
    wd[ii, R_RECON + ii] = GAMMA / (B * S * D)
    wd[ii, R_PTS + ii] = 1.0 / (B * S * PC)
    wd[ii, R_KL + ii] = BETA / (vector_dims * B * S)
    # marks region: per-batch [16,16] grams accumulated; diag p<8 = marks
    wd[ii, R_MARK + ii] = np.where(ii < NMARK * C, W_MARK / (B * S * PC), 0.0) * (
        ii < 16
    )

    base = (np.arange(BL, dtype=np.int32) * S)[:, None]

    in_maps = []
    for c in range(N_CORES):
        sl = slice(c * BL, (c + 1) * BL)
        map_abs = np.ascontiguousarray(
            (mapping[sl].astype(np.int32) + base).T
        )  # (S, BL)
        cpk = np.zeros((S, NCONST), dtype=np.float32)
        cpk[:, COL_MAP : COL_MAP + BL] = map_abs.view(np.float32)
        cpk[:, COL_BIAS] = np.float32(vector_dims * EPS)
        cpk[:P, COL_BEST : COL_BEST + BC] = (
            best_w[sl].transpose(1, 0, 2).reshape(P, BC)
        )
        cpk[:P, COL_BESTGT : COL_BESTGT + BC] = (
            bestgt_w[sl].transpose(1, 0, 2).reshape(P, BC)
        )
        in_maps.append(
            {
                # pre-transposed to the SBUF layout [S, BL*K]
                "neg": np.ascontiguousarray(
                    neg_b[sl].transpose(1, 0, 2).reshape(S, BL * K)
                ),
                "gath": gath_b[sl].reshape(BL * S, K),
                "qt": np.ascontiguousarray(
                    qt_b[sl].transpose(1, 0, 2).reshape(S, BL * V)
                ),
                "cpack": cpk,
                "wdiag": wd,
            }
        )
    return in_maps, vector_dims


def _combine(results) -> np.ndarray:
    total = np.float64(0.0)
    for r in results:
        por = r["po"].astype(np.float64)
        total += por[:, 0].sum()  # weighted recon+pts+kl+marks
        total += por[:P, 1].sum() / (B * PC)  # best
    return np.float32(total)


def kernel(**inputs) -> np.ndarray:
    from concourse.bass_utils import run_bass_kernel_spmd

    in_maps, vector_dims = _prepare(inputs)
    nc = _get_nc(vector_dims)

    trace = os.environ.get("KERNEL_TRACE", "") == "1"
    res = run_bass_kernel_spmd(nc, in_maps, core_ids=list(range(N_CORES)), trace=trace)
    if trace and res.exec_time_ns is not None:
        print(f"HW exec time: {res.exec_time_ns} ns")
        if res.instructions_and_trace is not None:
            print(f"trace: {res.instructions_and_trace[1]}")

    return _combine(res.results)


# revision 29
# speedup vs baseline: 1.0078x; 1.0078x over previous
"""Trainium2 Bass kernel for nn_CQLoss (composite loss function).

Strategy: pure data parallel over batch dim (64 batches -> 8 per core).

All large tensors travel as fp8 e4m3 (TRN FP8_EXP4 == ml_dtypes.float8_e4m3):
the kernel is DMA-bound and the DMA cost is out-side bytes, so fp8 halves the
bf16 baseline's traffic.  Quantization contributes ~2e-3 relative error on
the final scalar (tolerance 2e-2).

Per core:
  - d = (rz|pts)[mapping] - (zs|pts_gt): the host packs [-zs | -pts_gt] rows
    pre-transposed to the exact SBUF layout [S, BL*K] (plain DMA pairs
    elements linearly in AP order, so identical shapes on both sides make
    the pairing trivially correct), and [rzs | pts] as the gather source.
    neg is DMA'd in plainly (HWDGE); the indirect gather lands on top with
    compute_op=add, so the subtraction happens inside the DMA engines.
  - sum-of-squares via TensorE gram trick: matmul(chunk, chunk) accumulated
    into a PSUM region; the diagonal of the accumulated gram holds
    sum-over-partitions of squares per column, so trace == sum d^2.  All
    regions live in ONE psum bank as ONE accumulation group (hw `start`
    zeroes the bank, so interleaved groups in a bank clobber each other;
    a single group accumulating at different addresses is safe):
    recon [*,0:128], pts [*,128:256], kl [*,256:384], markx [*,384:512]
    (landmark P-indices host-permuted to the front of the PC block; markx
    grams cols D..D+16 of every batch, mask selects the 8 mark cols).
  - KL: qt = V*qy (fp8, SBUF layout), l = Ln(qt + V*eps) on ScalarE (fp8
    out), then gram(qt_chunk, l_chunk): trace == sum qt*ln(qt).
  - best: f32 via cpack, sqrt(w)-prescaled, DVE sub + ScalarE Square with
    accum_out (tiny).
  - extraction: one DVE tensor_mul of the psum bank against an f32
    weight-diagonal mask (term normalizations baked in, DMA'd during the
    tail shadow), then one tensor_scalar accumulate into an acc column;
    host sums in f64.

Raw bass (explicit semaphores): standalone wait_ge ops, cumulative counts on
per-stream semaphores (DMAs of one stream complete in FIFO order), explicit
self-waits for same-engine RAW pairs.
"""

import os
import sys

import numpy as np

for _p in ("/opt/trn_rl_repo", "/root/.axon_site/_ro/trn_rl_repo"):
    if os.path.isdir(_p) and _p not in sys.path:
        sys.path.insert(0, _p)

B, S, D, P, C, V = 64, 128, 2048, 118, 2, 512
PC = P * C  # 236
K = D + PC  # combined row width: 2284
N_CORES = 8
BL = B // N_CORES  # 8 batches per core
ALPHA, BETA, GAMMA, EPS = 10.0, 0.1, 1.0, 1e-20
MARKS = (0, 29, 88, 117)
NMARK = len(MARKS)  # 4 -> 8 cols (C=2), host-permuted to the front
W_MARK = ALPHA * PC / (NMARK * C)  # 295.0

# neg-chunk layout: (start_batch, n_batches); small leading chunks so the
# first gathers can start early
NCH = [(0, 1), (1, 1), (2, 2), (4, 2), (6, 2)]


def _chunk_of(b):
    return next(i for i, (bs, nb) in enumerate(NCH) if bs <= b < bs + nb)


# cpack layout (f32 cols): 0..7 mapping (int32 bits), 8 ln bias,
# 9..24 sqrt(w)*best, 25..40 sqrt(w)*best_gt
BC = BL * C  # 16
COL_MAP = 0
COL_BIAS = BL  # 8
COL_BEST = 9
COL_BESTGT = 9 + BC  # 25
NCONST = 9 + 2 * BC  # 41

# psum region column offsets (single bank, single accumulation group).
# recon/pts/mark are contiguous [0:272] so the late extraction is one short
# STT; kl sits at [384:512] and is extracted early (it closes mid-stream).
R_RECON, R_PTS, R_MARK, R_KL = 0, 128, 256, 384

# DoubleRow fp8 matmul: two 128-col k-tiles per pass (2x PE throughput)
USE_DR = True

# sync-engine DMA program: ("cp",) ("neg",chunk) ("qt",half) ("wd",)
# ("wg",val) = wait sem_g >= val, to let gather transfers interleave
SCHED = [
    ("neg", 0),
    ("neg", 1),
    ("cp",),
    ("neg", 2),
    ("neg", 3),
    ("neg", 4),
    ("qt", 0),
    ("qt", 1),
    ("wd",),
]

_CACHE: dict = {}


def _build_bass(vector_dims: int):
    import concourse.bass as bass
    from concourse import mybir

    f32 = mybir.dt.float32
    fp8 = mybir.dt.float8e4
    i32 = mybir.dt.int32
    Act = mybir.ActivationFunctionType
    Alu = mybir.AluOpType

    nc = bass.Bass()

    neg = nc.dram_tensor("neg", [S, BL * K], fp8, kind="ExternalInput")
    gath = nc.dram_tensor("gath", [BL * S, K], fp8, kind="ExternalInput")
    qt = nc.dram_tensor("qt", [S, BL * V], fp8, kind="ExternalInput")
    cpack = nc.dram_tensor("cpack", [S, NCONST], f32, kind="ExternalInput")
    wdiag = nc.dram_tensor("wdiag", [S, 512], f32, kind="ExternalInput")
    # acc col 0: weighted recon+pts+kl+marks; col 1 (rows 0:118): best
    po = nc.dram_tensor("po", [S, 3], f32, kind="ExternalOutput")

    from contextlib import ExitStack

    with ExitStack() as ctx:
        d_t = ctx.enter_context(nc.sbuf_tensor([S, BL * K], fp8))
        qt_t = ctx.enter_context(nc.sbuf_tensor([S, BL * V], fp8))
        l_t = ctx.enter_context(nc.sbuf_tensor([S, BL * V], fp8))
        cp_t = ctx.enter_context(nc.sbuf_tensor([S, NCONST], f32))
        wd_t = ctx.enter_context(nc.sbuf_tensor([S, 512], f32))
        bd_t = ctx.enter_context(nc.sbuf_tensor([S, BC], f32))
        ext_t = ctx.enter_context(nc.sbuf_tensor([S, 512], f32))
        acc_t = ctx.enter_context(nc.sbuf_tensor([S, 3], f32))
        ps = ctx.enter_context(nc.psum_tensor([S, 512], f32))

        sem_cp = ctx.enter_context(nc.semaphore("sem_cp"))
        sem_neg = ctx.enter_context(nc.semaphore("sem_neg"))
        sem_qt = ctx.enter_context(nc.semaphore("sem_qt"))
        sem_g = ctx.enter_context(nc.semaphore("sem_g"))
        sem_wd = ctx.enter_context(nc.semaphore("sem_wd"))
        sem_act = ctx.enter_context(nc.semaphore("sem_act"))
        sem_dve = ctx.enter_context(nc.semaphore("sem_dve"))
        sem_pe = ctx.enter_context(nc.semaphore("sem_pe"))
        sem_out = ctx.enter_context(nc.semaphore("sem_out"))
        block = ctx.enter_context(nc.Block(no_gpsimd_drain=True))

        d3 = d_t[:].rearrange("s (b k) -> s b k", b=BL)
        map_i = cp_t[:, COL_MAP : COL_MAP + BL].bitcast(i32)

        @block.sync
        def _(sync):
            qh = BL // 2 * V
            for tok in SCHED:
                kind = tok[0]
                if kind == "cp":
                    sync.dma_start(out=cp_t[:], in_=cpack[:]).then_inc(sem_cp, 16)
                elif kind == "neg":
                    bs, nb = NCH[tok[1]]
                    sync.dma_start(
                        out=d_t[:, bs * K : (bs + nb) * K],
                        in_=neg[:, bs * K : (bs + nb) * K],
                    ).then_inc(sem_neg, 16)
                elif kind == "qt":
                    sl = slice(0, qh) if tok[1] == 0 else slice(qh, BL * V)
                    sync.dma_start(out=qt_t[:, sl], in_=qt[:, sl]).then_inc(
                        sem_qt, 16
                    )
                elif kind == "wd":
                    sync.dma_start(out=wd_t[:], in_=wdiag[:]).then_inc(sem_wd, 16)
                elif kind == "wg":
                    sync.wait_ge(sem_g, tok[1])
            sync.wait_ge(sem_dve, 3)
            sync.wait_ge(sem_act, 3)
            sync.dma_start(out=po[:], in_=acc_t[:]).then_inc(sem_out, 16)
            sync.wait_ge(sem_out, 16)

        @block.gpsimd
        def _(gpsimd):
            gpsimd.wait_ge(sem_cp, 16)  # mapping loaded
            for b in range(BL):
                gpsimd.wait_ge(sem_neg, 16 * (_chunk_of(b) + 1))
                gpsimd.indirect_dma_start(
                    out=d_t[:, b * K : (b + 1) * K],
                    out_offset=None,
                    in_=gath[:],
                    in_offset=bass.IndirectOffsetOnAxis(
                        ap=map_i[:, b : b + 1], axis=0
                    ),
                    compute_op=Alu.add,
                ).then_inc(sem_g, 16)

        # single accumulation group across the whole bank: only the very
        # first matmul has start=True, only the very last has stop=True
        first_mm = [True]
        DR = mybir.MatmulPerfMode.DoubleRow

        def mm(out, lhsT, rhs, stop=False, dr=False):
            inst = nc.tensor.matmul(
                out=out,
                lhsT=lhsT,
                rhs=rhs,
                start=first_mm[0],
                stop=stop,
                perf_mode=DR if dr else None,
                skip_group_check=True,
            )
            first_mm[0] = False
            return inst

        def gram(region, a, b=None, dr=None):
            # square-gram of columns `a` (or cross-gram a x b) into `region`;
            # dr pairs two k-tiles per pass (DoubleRow; needs 128-wide tiles)
            if USE_DR if dr is None else dr:
                w = a.shape[-1] // 2
                a2 = a.rearrange("s (t c) -> s t c", t=2)
                b2 = a2 if b is None else b.rearrange("s (t c) -> s t c", t=2)
                return mm(ps[0:w, region : region + w], a2, b2, dr=True)
            w = a.shape[-1]
            return mm(ps[0:w, region : region + w], a, a if b is None else b)

        def d_grams(tensor, b):
            tensor.wait_ge(sem_g, 16 * (b + 1))
            rw = 256 if USE_DR else 128
            for i in range(D // rw):
                gram(R_RECON, d3[:, b, i * rw : (i + 1) * rw])
            # pts: DoubleRow rejects 118-wide tiles (ISA check), so plain grams
            gram(R_PTS, d3[:, b, D : D + 128], dr=False)
            gram(R_PTS, d3[:, b, D + 128 : D + PC], dr=False)
            # marks: first 16 cols of the pts block (8 landmark + 8 other);
            # wdiag keeps only p < 8.  rhs must have a single free dim, so
            # one small gram per batch, all accumulating into [16,16].
            return mm(
                ps[0:16, R_MARK : R_MARK + 16],
                d3[:, b, D : D + 16],
                d3[:, b, D : D + 16],
                stop=(b == BL - 1),
            )

        @block.tensor
        def _(tensor):
            for b in range(6):
                d_grams(tensor, b)
            # kl grams: ln halves are done by the time batch 5 is gathered
            kw = 256 if USE_DR else 128
            tensor.wait_ge(sem_act, 2)
            for c in range(BL * V // kw):
                kl_last = gram(
                    R_KL,
                    qt_t[:, c * kw : (c + 1) * kw],
                    l_t[:, c * kw : (c + 1) * kw],
                )
            kl_last.then_inc(sem_pe, 1)  # kl region values final
            for b in range(6, BL):
                last = d_grams(tensor, b)
            last.then_inc(sem_pe, 1)

        @block.scalar
        def _(scalar):
            scalar.wait_ge(sem_cp, 16)
            scalar.wait_ge(sem_qt, 16)
            nc.scalar.activation(
                l_t[:, : BL // 2 * V],
                qt_t[:, : BL // 2 * V],
                Act.Ln,
                bias=cp_t[:, COL_BIAS : COL_BIAS + 1],
                scale=1.0,
            ).then_inc(sem_act, 1)
            scalar.wait_ge(sem_qt, 32)
            nc.scalar.activation(
                l_t[:, BL // 2 * V :],
                qt_t[:, BL // 2 * V :],
                Act.Ln,
                bias=cp_t[:, COL_BIAS : COL_BIAS + 1],
                scale=1.0,
            ).then_inc(sem_act, 1)
            # best term: acc[0:118, 1] = per-partition sum(bd^2), in place
            scalar.wait_ge(sem_dve, 1)
            nc.scalar.activation(
                bd_t[0:P, :],
                bd_t[0:P, :],
                Act.Square,
                accum_out=acc_t[0:P, 1:2],
            ).then_inc(sem_act, 1)

        @block.vector
        def _(vector):
            # best diff: bd = sqrt(w)*(best - best_gt), f32
            vector.wait_ge(sem_cp, 16)
            nc.vector.tensor_sub(
                bd_t[:],
                cp_t[:, COL_BEST : COL_BEST + BC],
                cp_t[:, COL_BESTGT : COL_BESTGT + BC],
            ).then_inc(sem_dve, 1)  # 1
            # fused extraction: (psum * 1.0) * wdiag, per-partition accumulate.
            # kl closes mid-stream -> extract early; recon/pts/mark at the end.
            vector.wait_ge(sem_wd, 16)
            vector.wait_ge(sem_pe, 1)
            nc.vector.scalar_tensor_tensor(
                out=ext_t[:, R_KL:512],
                in0=ps[:, R_KL:512],
                scalar=1.0,
                in1=wd_t[:, R_KL:512],
                op0=Alu.mult,
                op1=Alu.mult,
                accum_out=acc_t[:, 2:3],
            ).then_inc(sem_dve, 1)  # 2
            vector.wait_ge(sem_pe, 2)
            nc.vector.scalar_tensor_tensor(
                out=ext_t[:, 0 : R_MARK + 16],
                in0=ps[:, 0 : R_MARK + 16],
                scalar=1.0,
                in1=wd_t[:, 0 : R_MARK + 16],
                op0=Alu.mult,
                op1=Alu.mult,
                accum_out=acc_t[:, 0:1],
            ).then_inc(sem_dve, 1)  # 3

    return nc


def _get_nc(vector_dims: int):
    key = ("nc", vector_dims)
    if key not in _CACHE:
        _CACHE[key] = _build_bass(vector_dims)
    return _CACHE[key]


def _prepare(inputs):
    import ml_dtypes

    fp8 = ml_dtypes.float8_e4m3

    zs = np.asarray(inputs["zs"], dtype=np.float32)
    rzs = np.asarray(inputs["rzs"], dtype=np.float32)
    pts = np.asarray(inputs["pts"], dtype=np.float32)
    pts_gt = np.asarray(inputs["pts_gt"], dtype=np.float32)
    qy = np.asarray(inputs["qy"], dtype=np.float32)
    best = np.asarray(inputs["best"], dtype=np.float64)
    best_gt = np.asarray(inputs["best_gt"], dtype=np.float64)
    mapping = np.asarray(inputs["mapping"])
    vector_dims = int(np.asarray(inputs["vector_dims"]))

    # landmark P-indices permuted to the front of the P axis
    perm = list(MARKS) + [p for p in range(P) if p not in MARKS]
    pts_p = pts[:, :, perm, :].reshape(B, S, PC)
    ptsgt_p = pts_gt[:, :, perm, :].reshape(B, S, PC)

    neg_b = np.empty((B, S, K), dtype=fp8)
    neg_b[:, :, :D] = (-zs).astype(fp8)
    neg_b[:, :, D:] = (-ptsgt_p).astype(fp8)
    gath_b = np.empty((B, S, K), dtype=fp8)
    gath_b[:, :, :D] = rzs.astype(fp8)
    gath_b[:, :, D:] = pts_p.astype(fp8)
    qt_b = (qy * vector_dims).astype(fp8)

    # sqrt of landmark weights for the best term (exact in f64)
    w_p = np.ones(P, dtype=np.float64)
    w_p[list(MARKS)] += W_MARK
    w_sq = np.sqrt(w_p)
    best_w = (best * w_sq[None, :, None]).astype(np.float32)
    bestgt_w = (best_gt * w_sq[None, :, None]).astype(np.float32)

    # weighted diagonal extraction mask (term normalizations baked in)
    wd = np.zeros((S, 512), dtype=np.float32)
    ii = np.arange(128)
    wd[ii, R_RECON + ii] = GAMMA / (B * S * D)
    wd[ii, R_PTS + ii] = 1.0 / (B * S * PC)
    wd[ii, R_KL + ii] = BETA / (vector_dims * B * S)
    # marks region: per-batch [16,16] grams accumulated; diag p<8 = marks
    wd[ii, R_MARK + ii] = np.where(ii < NMARK * C, W_MARK / (B * S * PC), 0.0) * (
        ii < 16
    )

    base = (np.arange(BL, dtype=np.int32) * S)[:, None]

    in_maps = []
    for c in range(N_CORES):
        sl = slice(c * BL, (c + 1) * BL)
        map_abs = np.ascontiguousarray(
            (mapping[sl].astype(np.int32) + base).T
        )  # (S, BL)
        cpk = np.zeros((S, NCONST), dtype=np.float32)
        cpk[:, COL_MAP : COL_MAP + BL] = map_abs.view(np.float32)
        cpk[:, COL_BIAS] = np.float32(vector_dims * EPS)
        cpk[:P, COL_BEST : COL_BEST + BC] = (
            best_w[sl].transpose(1, 0, 2).reshape(P, BC)
        )
        cpk[:P, COL_BESTGT : COL_BESTGT + BC] = (
            bestgt_w[sl].transpose(1, 0, 2).reshape(P, BC)
        )
        in_maps.append(
            {
                # pre-transposed to the SBUF layout [S, BL*K]
                "neg": np.ascontiguousarray(
                    neg_b[sl].transpose(1, 0, 2).reshape(S, BL * K)
                ),
                "gath": gath_b[sl].reshape(BL * S, K),
                "qt": np.ascontiguousarray(
                    qt_b[sl].transpose(1, 0, 2).reshape(S, BL * V)
                ),
                "cpack": cpk,
                "wdiag": wd,
            }
        )
    return in_maps, vector_dims


def _combine(results) -> np.ndarray:
    total = np.float64(0.0)
    for r in results:
        por = r["po"].astype(np.float64)
        total += por[:, 0].sum() + por[:, 2].sum()  # weighted terms
        total += por[:P, 1].sum() / (B * PC)  # best
    return np.float32(total)


def kernel(**inputs) -> np.ndarray:
    from concourse.bass_utils import run_bass_kernel_spmd

    in_maps, vector_dims = _prepare(inputs)
    nc = _get_nc(vector_dims)

    trace = os.environ.get("KERNEL_TRACE", "") == "1"
    res = run_bass_kernel_spmd(nc, in_maps, core_ids=list(range(N_CORES)), trace=trace)
    if trace and res.exec_time_ns is not None:
        print(f"HW exec time: {res.exec_time_ns} ns")
        if res.instructions_and_trace is not None:
            print(f"trace: {res.instructions_and_trace[1]}")

    return _combine(res.results)
